# revision 18
# baseline (speedup 1.0000x reference)
"""CWCFace head (nn_CWCFace_11201274708637) — Trainium2 Bass kernel.

Math (reference):
    kn = kernel / ||kernel||_col
    cos = clip(emb @ kn, -1+eps, 1-eps)              # [B, C]
    ms  = margin_scaler(norms, label)                # [B, 1] per-sample stats
    th  = arccos(cos); th_m = clip(th + onehot*(-M*ms), eps, pi-eps)
    out = (cos(th_m) - onehot*(M + M*ms)) * S

Key observation: the onehot terms touch exactly ONE column per row, so the
full [B, C] tensor only needs  out = S * cos  plus a B-element fix-up at
(i, label_i).  cos(th+g) is evaluated for those B elements with the identity
cos(th+g) = t*cos(g) - sqrt(1-t^2)*sin(g) (small-angle g in [-0.4, 0.4]),
with the clip branches handled by threshold comparisons — no arccos needed.

Sharding: classes column-split over 8 cores (model-parallel ArcFace).
Per core: Cs = 8960 classes (total padded to 71680).  Inside each core:
  - f32r (TF32-like) matmuls: out^T tiles [B=128, W<=512] over 4 K-tiles
  - column norms via Square(ACT) + ones-vector matmul (PE partition reduce),
    1/sqrt via Exp(-0.5*Ln) on ACT
  - epilogue on DVE: out = (psum * S) * colscale  (scalar_tensor_tensor)
  - margin stats via BxB label-equality matmul; fix-up via indirect
    gather / scatter DMA (single f32 per sample) with OOB-skip for labels
    owned by other cores.
"""

import sys

for _p in (
    "/root/.axon_site",
    "/root/.axon_site/_ro/trn_rl_repo",
    "/root/.axon_site/_ro/pypackages",
    "/opt/trn_rl_repo",
):
    if _p not in sys.path:
        sys.path.append(_p)

import math

import numpy as np

import concourse.bass as bass
import concourse.mybir as mybir
import concourse.tile as tile
from concourse import bacc
from concourse.bass import IndirectOffsetOnAxis
from concourse.bass_utils import run_bass_kernel_spmd

B = 512
EMB = 512
C = 70722
NCORES = 8
CS = 8960  # per-core classes (padded);  8 * 8960 = 71680 >= 70722
S = 64.0
MARG = 0.4
H = 0.333
EPS = 1e-3

F32 = mybir.dt.float32
F32R = mybir.dt.float32r
I32 = mybir.dt.int32
AL = mybir.AluOpType
AF = mybir.ActivationFunctionType

KT = EMB // 128          # 4 K-tiles
BT = B // 128            # 4 B-tiles
COS_EPS = float(math.cos(EPS))
PI_2 = math.pi / 2.0


def _slices():
    """Class-column slices per core: widths <=512, all >=256 (f32r full rate)."""
    out = []
    c0 = 0
    while c0 < CS:
        w = min(512, CS - c0)
        out.append((c0, w))
        c0 += w
    return out


def _emit(nc, tc, embT_h, kern_h, lab_h, nrm_h, ones_h, out_h):
    out2d = out_h[:, :].rearrange("(r c) o -> r (c o)", c=CS)  # [B, CS] view

    cst_cm = tc.tile_pool(name="cst", bufs=1)
    cst = cst_cm.__enter__()

    # ---- constants / persistent tiles -------------------------------------
    embT_sb = cst.tile([128, KT, B], F32R, tag="embT")  # [p, k, b]
    nc.sync.dma_start(
        out=embT_sb[:], in_=embT_h[:, :].rearrange("(k p) b -> p k b", p=128)
    )
    lab_sb = cst.tile([128, BT], I32, tag="lab")
    nc.sync.dma_start(
        out=lab_sb[:], in_=lab_h[:, :].rearrange("(b p) o -> p (b o)", p=128)
    )
    nrm_sb = cst.tile([128, BT], F32, tag="nrm")
    nc.sync.dma_start(
        out=nrm_sb[:], in_=nrm_h[:, :].rearrange("(b p) o -> p (b o)", p=128)
    )
    labrow = cst.tile([1, B], I32, tag="labrow")
    nc.sync.dma_start(out=labrow[:], in_=lab_h[:, :].rearrange("b o -> o b"))

    ones_col = cst.tile([128, 1], F32R, tag="ones_col")
    nc.sync.dma_start(out=ones_col[:], in_=ones_h[:, 0:1])
    ones_k1 = cst.tile([1, 128], F32, tag="ones_k1")
    nc.vector.memset(ones_k1[:], 1.0)
    ones_k1r = cst.tile([1, 128], F32R, tag="ones_k1r")
    nc.sync.dma_start(out=ones_k1r[:], in_=ones_h[0:1, :])

    # persistent per-sample results of phase A
    g_sb = cst.tile([128, BT], F32, tag="g")        # -M * ms
    gadd_sb = cst.tile([128, BT], F32, tag="gadd")  # M + M * ms
    v_sb = cst.tile([128, BT], F32, tag="v")        # safe norms

    # =======================================================================
    # Phase A: margin scaler stats (tiny, [B] sized)
    # =======================================================================
    with (
        tc.tile_pool(name="pa", bufs=2) as pa,
        tc.tile_pool(name="psA", bufs=2, space="PSUM") as psA,
    ):
        lab_f = pa.tile([128, BT], F32, tag="lab_f")
        nc.vector.tensor_copy(lab_f[:], lab_sb[:])
        labrow_f = pa.tile([1, B], F32, tag="labrow_f")
        nc.vector.tensor_copy(labrow_f[:], labrow[:])

        # broadcast full label row to all partitions via K=1 matmul
        ps_lr = psA.tile([128, B], F32, space="PSUM", tag="lr")
        nc.tensor.matmul(ps_lr[:], ones_k1[:], labrow_f[:], start=True, stop=True)
        labAll = pa.tile([128, B], F32, tag="labAll")
        nc.scalar.copy(labAll[:], ps_lr[:])

        # safe norms and [1, v, v^2] stats rhs
        nc.vector.tensor_scalar(
            v_sb[:], nrm_sb[:], 0.001, 100.0, op0=AL.max, op1=AL.min
        )
        w_sb = pa.tile([128, 3 * BT], F32, tag="w")
        nc.vector.memset(w_sb[:], 1.0)
        for b in range(BT):
            nc.vector.tensor_copy(w_sb[:, 3 * b + 1 : 3 * b + 2], v_sb[:, b : b + 1])
            nc.vector.tensor_tensor(
                out=w_sb[:, 3 * b + 2 : 3 * b + 3],
                in0=v_sb[:, b : b + 1],
                in1=v_sb[:, b : b + 1],
                op=AL.mult,
            )

        for a in range(BT):
            ps_st = psA.tile([128, 3], F32, space="PSUM", tag="st")
            for b in range(BT):
                eq = pa.tile([128, 128], F32, tag="eq")
                nc.vector.tensor_tensor(
                    out=eq[:],
                    in0=lab_f[:, b : b + 1].to_broadcast([128, 128]),
                    in1=labAll[:, a * 128 : (a + 1) * 128],
                    op=AL.is_equal,
                )
                nc.tensor.matmul(
                    ps_st[:],
                    eq[:],
                    w_sb[:, 3 * b : 3 * b + 3],
                    start=(b == 0),
                    stop=(b == BT - 1),
                )
            st = pa.tile([128, 3], F32, tag="stc")
            nc.vector.tensor_copy(st[:], ps_st[:])
            n_ = st[:, 0:1]
            sm = st[:, 1:2]
            sq2 = st[:, 2:3]

            t0 = pa.tile([128, 8], F32, tag="t0")  # scratch columns
            rn = t0[:, 0:1]
            nc.vector.reciprocal(rn, n_)
            mean = t0[:, 1:2]
            nc.vector.tensor_tensor(out=mean, in0=sm, in1=rn, op=AL.mult)
            m2 = t0[:, 2:3]
            nc.vector.tensor_tensor(out=m2, in0=mean, in1=mean, op=AL.mult)
            nm2 = t0[:, 3:4]
            nc.vector.tensor_tensor(out=nm2, in0=n_, in1=m2, op=AL.mult)
            num = t0[:, 4:5]
            nc.vector.tensor_tensor(out=num, in0=sq2, in1=nm2, op=AL.subtract)
            den = t0[:, 5:6]
            nc.vector.tensor_scalar(den, n_, -1.0, 1.0, op0=AL.add, op1=AL.max)
            rden = t0[:, 6:7]
            nc.vector.reciprocal(rden, den)
            var = t0[:, 7:8]
            nc.vector.tensor_tensor(out=var, in0=num, in1=rden, op=AL.mult)
            nc.vector.tensor_scalar(var, var, 1e-30, None, op0=AL.max)

            t1 = pa.tile([128, 8], F32, tag="t1")
            lnv = t1[:, 0:1]
            nc.scalar.activation(lnv, var, AF.Ln)
            std = t1[:, 1:2]
            nc.scalar.activation(std, lnv, AF.Exp, scale=0.5)  # sqrt(var)
            stdp = t1[:, 2:3]
            nc.vector.tensor_scalar(stdp, std, EPS, None, op0=AL.add)
            rstd = t1[:, 3:4]
            nc.vector.reciprocal(rstd, stdp)
            mask = t1[:, 4:5]
            nc.vector.tensor_scalar(mask, n_, 2.0, None, op0=AL.is_gt)
            mask_i = pa.tile([128, 1], I32, tag="mask_i")
            nc.vector.tensor_copy(mask_i[:], mask)
            c05 = t1[:, 5:6]
            nc.vector.memset(c05, 0.05)
            invd = t1[:, 6:7]
            nc.vector.select(invd, mask_i[:], rstd, c05)
            dv = t1[:, 7:8]
            nc.vector.tensor_tensor(
                out=dv, in0=v_sb[:, a : a + 1], in1=mean, op=AL.subtract
            )
            res = t0[:, 0:1]  # reuse
            nc.vector.tensor_tensor(out=res, in0=dv, in1=invd, op=AL.mult)
            ms = t0[:, 1:2]
            nc.vector.tensor_scalar(ms, res, H, 1.0, op0=AL.mult, op1=AL.min)
            nc.vector.tensor_scalar(ms, ms, -1.0, None, op0=AL.max)
            nc.vector.tensor_scalar(
                g_sb[:, a : a + 1], ms, -MARG, None, op0=AL.mult
            )
            nc.vector.tensor_scalar(
                gadd_sb[:, a : a + 1], ms, MARG, MARG, op0=AL.mult, op1=AL.add
            )

    # =======================================================================
    # Phase B: main model-parallel matmul + column-norm epilogue
    # =======================================================================
    kernR = kern_h[:, :].rearrange("(k p) c -> p k c", p=128)  # [128, KT, CS]

    with (
        tc.tile_pool(name="kp", bufs=3) as kp,
        tc.tile_pool(name="wp", bufs=2) as wp,
        tc.tile_pool(name="op", bufs=8) as op_,
        tc.tile_pool(name="ps_o", bufs=4, space="PSUM") as ps_o,
        tc.tile_pool(name="ps_m", bufs=2, space="PSUM") as ps_m,
    ):
        for c0, W in _slices():
            ks = kp.tile([128, KT, W], F32R, tag="ks")
            nc.sync.dma_start(out=ks[:], in_=kernR[:, :, c0 : c0 + W])

            # column sums of squares -> [1, W] via ones-matmul over K
            ksq = wp.tile([128, KT * W], F32R, tag="ksq")
            for k in range(KT):
                nc.scalar.activation(
                    ksq[:, k * W : (k + 1) * W], ks[:, k, :].bitcast(F32), AF.Square
                )
            ps_ssq = ps_m.tile([1, W], F32, space="PSUM", tag="ssq")
            for k in range(KT):
                nc.tensor.matmul(
                    ps_ssq[:],
                    ones_col[:],
                    ksq[:, k * W : (k + 1) * W],
                    start=(k == 0),
                    stop=(k == KT - 1),
                )
            lnrow = wp.tile([1, W], F32, tag="lnrow")
            nc.scalar.activation(lnrow[:], ps_ssq[:], AF.Ln)
            invrow = wp.tile([1, W], F32R, tag="invrow")
            nc.scalar.activation(invrow[:], lnrow[:], AF.Exp, scale=-0.5)

            # broadcast col scale to 128 partitions via K=1 matmul
            ps_bc = ps_m.tile([128, W], F32, space="PSUM", tag="bc")
            nc.tensor.matmul(
                ps_bc[:],
                ones_k1r[:],
                invrow[:],
                start=True,
                stop=True,
            )
            scale_bc = wp.tile([128, W], F32, tag="scale_bc")
            nc.scalar.copy(scale_bc[:], ps_bc[:])

            for b in range(BT):
                ps_out = ps_o.tile([128, W], F32, space="PSUM", tag="po")
                for k in range(KT):
                    nc.tensor.matmul(
                        ps_out[:],
                        embT_sb[:, k, b * 128 : (b + 1) * 128],
                        ks[:, k, :],
                        start=(k == 0),
                        stop=(k == KT - 1),
                    )
                o_sb = op_.tile([128, W], F32, tag="o")
                nc.vector.scalar_tensor_tensor(
                    out=o_sb[:],
                    in0=ps_out[:],
                    scalar=S,
                    in1=scale_bc[:],
                    op0=AL.mult,
                    op1=AL.mult,
                )
                # cosine clip (reference clips to [-1+eps, 1-eps] pre-arccos)
                nc.vector.tensor_scalar(
                    o_sb[:],
                    o_sb[:],
                    -S * (1.0 - EPS),
                    S * (1.0 - EPS),
                    op0=AL.max,
                    op1=AL.min,
                )
                nc.sync.dma_start(
                    out=out2d[b * 128 : (b + 1) * 128, c0 : c0 + W], in_=o_sb[:]
                )

    # =======================================================================
    # Phase C: per-sample label-column fix-up (gather -> math -> scatter)
    # =======================================================================
    tc.strict_bb_all_engine_barrier()

    with tc.tile_pool(name="pc", bufs=1) as pc:
        # indices
        rb = pc.tile([128, BT], I32, tag="rb")
        for b in range(BT):
            nc.gpsimd.iota(
                rb[:, b : b + 1],
                pattern=[[0, 1]],
                base=b * 128 * CS,
                channel_multiplier=CS,
            )
        ccl = pc.tile([128, BT], I32, tag="ccl")
        nc.vector.tensor_scalar(ccl[:], lab_sb[:], 0, CS - 1, op0=AL.max, op1=AL.min)
        gidx = pc.tile([128, BT], I32, tag="gidx")
        nc.vector.tensor_tensor(out=gidx[:], in0=rb[:], in1=ccl[:], op=AL.add)

        # in-range mask (labels owned by this core) and scatter index
        mi1 = pc.tile([128, BT], I32, tag="mi1")
        nc.vector.tensor_scalar(mi1[:], lab_sb[:], 0, None, op0=AL.is_ge)
        mi2 = pc.tile([128, BT], I32, tag="mi2")
        nc.vector.tensor_scalar(mi2[:], lab_sb[:], CS, None, op0=AL.is_lt)
        mi = pc.tile([128, BT], I32, tag="mi")
        nc.vector.tensor_tensor(out=mi[:], in0=mi1[:], in1=mi2[:], op=AL.mult)
        # sidx = gidx + (1 - mi) * 2^30
        off = pc.tile([128, BT], I32, tag="off")
        nc.vector.tensor_scalar(
            off[:], mi[:], -(2**30), 2**30, op0=AL.mult, op1=AL.add
        )
        sidx = pc.tile([128, BT], I32, tag="sidx")
        nc.vector.tensor_tensor(out=sidx[:], in0=gidx[:], in1=off[:], op=AL.add)

        # gather current S*cos at (i, label_i)
        traw = pc.tile([128, BT], F32, tag="traw")
        for b in range(BT):
            nc.gpsimd.indirect_dma_start(
                out=traw[:, b : b + 1],
                out_offset=None,
                in_=out_h[:, :],
                in_offset=IndirectOffsetOnAxis(ap=gidx[:, b : b + 1], axis=0),
            )

        t_ = pc.tile([128, BT], F32, tag="t_")
        nc.vector.tensor_scalar(
            t_[:], traw[:], 1.0 / S, 1.0 - EPS, op0=AL.mult, op1=AL.min
        )
        nc.vector.tensor_scalar(t_[:], t_[:], -1.0 + EPS, None, op0=AL.max)

        # const bias tiles for activation calls (float biases need const APs)
        cpi2 = pc.tile([128, 1], F32, tag="cpi2")
        nc.vector.memset(cpi2[:], PI_2)
        cpie = pc.tile([128, 1], F32, tag="cpie")
        nc.vector.memset(cpie[:], PI_2 + EPS)
        cone = pc.tile([128, 1], F32, tag="cone")
        nc.vector.memset(cone[:], 1.0)

        # trig of the small angle g (all Sin-set ACT ops batched)
        cosg = pc.tile([128, BT], F32, tag="cosg")
        sing = pc.tile([128, BT], F32, tag="sing")
        thr_lo = pc.tile([128, BT], F32, tag="thr_lo")
        thr_hi = pc.tile([128, BT], F32, tag="thr_hi")
        for b in range(BT):
            gb = g_sb[:, b : b + 1]
            nc.scalar.activation(cosg[:, b : b + 1], gb, AF.Sin, bias=cpi2[:])
            nc.scalar.activation(sing[:, b : b + 1], gb, AF.Sin)
            # cos(eps - g) = sin(pi/2 + eps - g)
            nc.scalar.activation(
                thr_lo[:, b : b + 1], gb, AF.Sin, bias=cpie[:], scale=-1.0
            )
            # cos(eps + g) = sin(pi/2 + eps + g)
            nc.scalar.activation(
                thr_hi[:, b : b + 1], gb, AF.Sin, bias=cpie[:], scale=1.0
            )
        nthr = pc.tile([128, BT], F32, tag="nthr")
        nc.vector.tensor_scalar(nthr[:], thr_hi[:], -1.0, None, op0=AL.mult)

        # sqrt(1 - t^2)
        t2 = pc.tile([128, BT], F32, tag="t2")
        nc.scalar.activation(t2[:], t_[:], AF.Square)
        sq = pc.tile([128, BT], F32, tag="sq")
        nc.scalar.activation(sq[:], t2[:], AF.Sqrt, scale=-1.0, bias=cone[:])

        # cos(th + g) = t*cos(g) - sq*sin(g)
        a1 = pc.tile([128, BT], F32, tag="a1")
        nc.vector.tensor_tensor(out=a1[:], in0=t_[:], in1=cosg[:], op=AL.mult)
        a2 = pc.tile([128, BT], F32, tag="a2")
        nc.vector.tensor_tensor(out=a2[:], in0=sq[:], in1=sing[:], op=AL.mult)
        cosm = pc.tile([128, BT], F32, tag="cosm")
        nc.vector.tensor_tensor(out=cosm[:], in0=a1[:], in1=a2[:], op=AL.subtract)

        # clip branches
        ml1 = pc.tile([128, BT], F32, tag="ml1")
        nc.vector.tensor_scalar(ml1[:], g_sb[:], EPS, None, op0=AL.is_lt)
        ml2 = pc.tile([128, BT], F32, tag="ml2")
        nc.vector.tensor_tensor(out=ml2[:], in0=t_[:], in1=thr_lo[:], op=AL.is_gt)
        mlow = pc.tile([128, BT], F32, tag="mlow")
        nc.vector.tensor_tensor(out=mlow[:], in0=ml1[:], in1=ml2[:], op=AL.mult)
        mh1 = pc.tile([128, BT], F32, tag="mh1")
        nc.vector.tensor_scalar(mh1[:], g_sb[:], -EPS, None, op0=AL.is_gt)
        mh2 = pc.tile([128, BT], F32, tag="mh2")
        nc.vector.tensor_tensor(out=mh2[:], in0=t_[:], in1=nthr[:], op=AL.is_lt)
        mhigh = pc.tile([128, BT], F32, tag="mhigh")
        nc.vector.tensor_tensor(out=mhigh[:], in0=mh1[:], in1=mh2[:], op=AL.mult)

        mlow_i = pc.tile([128, BT], I32, tag="mlow_i")
        nc.vector.tensor_copy(mlow_i[:], mlow[:])
        mhigh_i = pc.tile([128, BT], I32, tag="mhigh_i")
        nc.vector.tensor_copy(mhigh_i[:], mhigh[:])
        c_lo = pc.tile([128, BT], F32, tag="c_lo")
        nc.vector.memset(c_lo[:], COS_EPS)
        c_hi = pc.tile([128, BT], F32, tag="c_hi")
        nc.vector.memset(c_hi[:], -COS_EPS)
        nc.vector.select(cosm[:], mlow_i[:], c_lo[:], cosm[:])
        nc.vector.select(cosm[:], mhigh_i[:], c_hi[:], cosm[:])

        # final value and scatter
        val = pc.tile([128, BT], F32, tag="val")
        nc.vector.tensor_tensor(out=val[:], in0=cosm[:], in1=gadd_sb[:], op=AL.subtract)
        nc.vector.tensor_scalar(val[:], val[:], S, None, op0=AL.mult)

        for b in range(BT):
            nc.gpsimd.indirect_dma_start(
                out=out_h[:, :],
                out_offset=IndirectOffsetOnAxis(ap=sidx[:, b : b + 1], axis=0),
                in_=val[:, b : b + 1],
                in_offset=None,
                bounds_check=B * CS - 1,
                oob_is_err=False,
            )

    cst_cm.__exit__(None, None, None)


def _build():
    nc = bacc.Bacc(
        "TRN2", target_bir_lowering=False, debug=False, num_devices=NCORES
    )
    embT_h = nc.dram_tensor("embT", [EMB, B], F32R, kind="ExternalInput")
    kern_h = nc.dram_tensor("kern", [EMB, CS], F32R, kind="ExternalInput")
    lab_h = nc.dram_tensor("lab", [B, 1], I32, kind="ExternalInput")
    nrm_h = nc.dram_tensor("nrm", [B, 1], F32, kind="ExternalInput")
    ones_h = nc.dram_tensor("ones", [128, 128], F32R, kind="ExternalInput")
    out_h = nc.dram_tensor("out", [B * CS, 1], F32, kind="ExternalOutput")
    with tile.TileContext(nc) as tc:
        _emit(nc, tc, embT_h, kern_h, lab_h, nrm_h, ones_h, out_h)
    nc.compile()
    return nc


_NC = None


def _get_nc():
    global _NC
    if _NC is None:
        _NC = _build()
    return _NC


def _prep_inputs(embbedings, norms, label, kernel):
    embT = np.ascontiguousarray(np.asarray(embbedings, dtype=np.float32).T)
    nrm = np.asarray(norms, dtype=np.float32).reshape(B, 1)
    lab = np.asarray(label).astype(np.int64).reshape(B)
    kern = np.asarray(kernel, dtype=np.float32)
    kern_pad = np.ones((EMB, CS * NCORES), dtype=np.float32)
    kern_pad[:, :C] = kern
    in_maps = []
    for c in range(NCORES):
        lab_adj = (lab - c * CS).astype(np.int32).reshape(B, 1)
        in_maps.append(
            {
                "embT": embT,
                "kern": np.ascontiguousarray(kern_pad[:, c * CS : (c + 1) * CS]),
                "lab": lab_adj,
                "nrm": nrm,
                "ones": np.ones((128, 128), dtype=np.float32),
            }
        )
    return in_maps


def _run(in_maps, **kwargs):
    nc = _get_nc()
    return run_bass_kernel_spmd(nc, in_maps, core_ids=list(range(NCORES)), **kwargs)


def kernel(embbedings, norms, label, kernel):
    in_maps = _prep_inputs(embbedings, norms, label, kernel)
    res = _run(in_maps)
    parts = [res.results[c]["out"].reshape(B, CS) for c in range(NCORES)]
    return np.concatenate(parts, axis=1)[:, :C].astype(np.float32)
